# revision 43
# baseline (speedup 1.0000x reference)
"""DeepFactor (K relu-LSTM branches + shared Dense head) on 8 trn2 NeuronCores.

Sharding: the K=10 factor branches are expert-split across cores, 2 slots
per core (16 slots = 10 real + 6 zero-padded; zero weights keep the padded
slot's state identically 0 so padding is exact). Every core runs the same
SPMD program over the full batch B=32.

Time-segment parallelism: the recurrence is latency-bound (the per-step
h -> matmul -> sigmoid -> gates -> h chain is ~1.5-2us while every engine
is far from busy), so each core runs SEGS independent time segments
concurrently. The LSTM is strongly contractive (unit forget bias => forget
gate ~0.7, so state memory decays ~0.7^t): segment j>0 restarts from zero
state WARMUP steps early, and after the warmup its trajectory has converged
to the exact one (L=32 => rel err ~1e-4 measured against the fp32
reference; tol is 2e-2). Serial chain length drops from T to T/SEGS+WARMUP.

Group fusion: chains are fused in groups of GRP for the elementwise ops.
A group shares one PSUM z tile laid out gate-major across chains
([i*GRP | f*GRP | o*GRP | c*GRP] blocks of 32 batch cols each), so the
sigmoid / t1 / t2 / add / h ops each process GRP chains in ONE instruction,
amortizing the per-op fixed costs (DVE access-init 60-125ns, Pool Q7
launch 95ns, ACT init 185ns, SEQ decode) across GRP chains.

Engine split per group-step:
  PE :  z_g = LX_g.T @ [x_t;1] (start) + LH_g.T @ h (stop), per gate/chain
  ACT:  sig = sigmoid(z[:, i|f|o blocks])   exact, one op per group
  DVE:  t1  = max(zc, 0) * sig_i            scalar_tensor_tensor, one op
  Pool: t2  = sig_f * c                     one op
  P/D :  c'  = t1 + t2                      add split across Pool/DVE
  DVE:  h   = sig_o * c'                    relu(c') == c' since c' >= 0
  PE :  y_t = h_j.T @ [Wd;Wd]               per chain, one PSUM column
All chains run exactly T/SEGS + WARMUP steps (chain 0 runs WARMUP extra
steps at its tail instead of a head warmup; y is only emitted for steps
inside the chain's own output segment). Host: y = (sum of cores)/K + bd.
"""

import os
from contextlib import ExitStack

import numpy as np

import concourse.bass as bass
import concourse.tile as tile
from concourse import bacc, mybir
from concourse.alu_op_type import AluOpType
from concourse.bass_utils import run_bass_kernel_spmd

# Problem dims (hardcoded per contract)
B, T, D, U, K = 32, 1024, 32, 64, 10
NCORES = 8

FP16 = os.environ.get("KERNEL_FP16", "1") == "1"
SEGS = int(os.environ.get("KERNEL_SEGS", "20"))      # parallel time segments
GRP = int(os.environ.get("KERNEL_GRP", "4"))         # chains fused per group
WARMUP = int(os.environ.get("KERNEL_WARMUP", "15"))  # zero-state warmup steps
SIG_BUFS = int(os.environ.get("KERNEL_SIG_BUFS", "2"))
# split the sigmoid into i|f (critical: feeds t1/t2) and o (feeds h, late)
SIG_SPLIT = os.environ.get("KERNEL_SIG_SPLIT", "0") == "1"
# fp16 elementwise state (sig/c/t1/t2): halves DVE cycles via 2x mode
FP16_STATE = os.environ.get("KERNEL_FP16_STATE", "1") == "1"
ROT = os.environ.get("KERNEL_ROT", "0") == "1"  # rotate group emission order
TWO_PASS = os.environ.get("KERNEL_TWO_PASS", "0") == "1"
H_POOL_GROUPS = int(os.environ.get("KERNEL_H_POOL_GROUPS", "0"))
Y_DIRECT = os.environ.get("KERNEL_Y_DIRECT", "0") == "1"  # DMA y straight from PSUM
X_SPLIT_ENG = os.environ.get("KERNEL_X_SPLIT_ENG", "1") == "1"  # x chunks on 2 DMA paths  # defer add/h ops
# number of groups whose c'=t1+t2 add runs on DVE instead of Pool (balance)
ADD_DVE_GROUPS = int(os.environ.get("KERNEL_ADD_DVE_GROUPS", "5"))

D_AUG = D + 1  # x rows + bias row

# gate order in the reference weights (Keras): i|f|c|o
_REF_GATE_SLICE = {"i": 0, "f": 1, "c": 2, "o": 3}
# our gate order: i|f|o (sigmoid block) then c (relu'd candidate)
_OUR_GATES = ["i", "f", "o", "c"]


def _np_dt():
    return np.float16 if FP16 else np.float32


def _mm_dt():
    return mybir.dt.float16 if FP16 else mybir.dt.float32


def _segments(t_steps):
    """Per-chain (start, out0, end). All chains run the same number of
    steps n = seg + WARMUP. Chains j>0 warm up from zero state for WARMUP
    steps before their output segment; chain 0 needs no warmup so its
    output segment is the full n steps (y emission is masked outside
    [out0, end))."""
    seg = -(-(t_steps - WARMUP) // SEGS)
    n = seg + WARMUP
    chains = []
    pos = 0
    for j in range(SEGS):
        out0 = pos
        end = min(t_steps, out0 + (n if j == 0 else seg))
        if out0 >= end:
            continue
        start = max(0, out0 - WARMUP)
        chains.append((start, out0, end))
        pos = end
    assert pos >= t_steps
    return chains, n


def _build_core_inputs(x, W, U_rec, b, Wd):
    """Per-core numpy input dicts. Slot assignment: core0:(k0,k1), core1:(k2,k3),
    cores 2-7: (k4+i, pad)."""
    ndt = _np_dt()
    xt = np.ascontiguousarray(np.transpose(x, (2, 1, 0)).reshape(D, T * B))
    xaug = np.concatenate(
        [xt, np.ones((1, T * B), np.float32)], axis=0
    ).astype(ndt)

    slot_ks = [(0, 1), (2, 3)] + [(4 + i, None) for i in range(6)]

    in_maps = []
    for core in range(NCORES):
        ks = slot_ks[core]
        LX = np.zeros((4, D_AUG, 2 * U), np.float32)  # [gate, 33, 128]
        LH = np.zeros((4, 2 * U, 2 * U), np.float32)  # [gate, 128, 128] blockdiag
        WD2 = np.zeros((2 * U, 1), np.float32)
        for s, k in enumerate(ks):
            if k is None:
                continue
            for g, gname in enumerate(_OUR_GATES):
                ref_g = _REF_GATE_SLICE[gname]
                cols = slice(ref_g * U, (ref_g + 1) * U)
                LX[g, :D, s * U:(s + 1) * U] = W[k][:, cols]
                LX[g, D, s * U:(s + 1) * U] = b[k][cols]
                LH[g, s * U:(s + 1) * U, s * U:(s + 1) * U] = U_rec[k][:, cols]
            WD2[s * U:(s + 1) * U, 0] = Wd[:, 0]
        in_maps.append(
            {
                "xaug": xaug,
                "lx": np.ascontiguousarray(
                    np.concatenate(list(LX), axis=1).astype(ndt)),
                "lh": np.ascontiguousarray(
                    np.concatenate(list(LH), axis=1).astype(ndt)),
                "wd2": WD2.astype(ndt),
            }
        )
    return in_maps


def _build_program(t_steps: int) -> bacc.Bacc:
    nc = bacc.Bacc(
        "TRN2",
        target_bir_lowering=False,
        debug=False,
        enable_asserts=False,
        num_devices=NCORES,
    )
    MDT = _mm_dt()
    F32 = mybir.dt.float32
    SDT = mybir.dt.float16 if FP16_STATE else F32
    SIGMOID = mybir.ActivationFunctionType.Sigmoid
    xaug_ap = nc.dram_tensor("xaug", [D_AUG, T * B], MDT, kind="ExternalInput").ap()
    lx_ap = nc.dram_tensor("lx", [D_AUG, 4 * 2 * U], MDT, kind="ExternalInput").ap()
    lh_ap = nc.dram_tensor("lh", [2 * U, 4 * 2 * U], MDT, kind="ExternalInput").ap()
    wd2_ap = nc.dram_tensor("wd2", [2 * U, 1], MDT, kind="ExternalInput").ap()
    y_ap = nc.dram_tensor("y", [B, t_steps], F32, kind="ExternalOutput").ap()

    P = 2 * U  # 128
    chains, n_steps = _segments(t_steps)
    NCH = len(chains)
    assert NCH == SEGS, "partial tail segment not supported by grouping"
    NG = -(-NCH // GRP)
    GW = GRP * B          # fused elementwise width per group

    with tile.TileContext(nc) as tc, ExitStack() as ctx:
        const_pool = ctx.enter_context(tc.tile_pool(name="const", bufs=1))
        state_pool = ctx.enter_context(tc.tile_pool(name="state", bufs=1))
        zst_pool = ctx.enter_context(
            tc.tile_pool(name="zst", bufs=1, space="PSUM")
        )
        sig_pools = [
            ctx.enter_context(tc.tile_pool(name=f"sg{g}", bufs=SIG_BUFS))
            for g in range(NG)
        ]
        ypsum_pool = ctx.enter_context(tc.tile_pool(name="yps", bufs=1, space="PSUM"))
        out_pool = ctx.enter_context(tc.tile_pool(name="out", bufs=1))

        # --- static weights + full x into SBUF ---
        # Weights first: every matmul needs them, and the serialized DMA
        # channel delivers in issue order (x-first measurably regresses).
        DMA_ENG = {"gpsimd": nc.gpsimd, "sync": nc.sync, "scalar": nc.scalar}[
            os.environ.get("KERNEL_DMA_ENG", "sync")
        ]
        # gates packed side by side: one DMA for all x-weights, one for all
        # rec-weights. The two DMA initiation paths (gpsimd SWDGE / ACT
        # HWDGE) run in parallel, so balance bytes across them: big lh +
        # one x half on path A, small lx + wd2 + other x half on path B.
        lxall = const_pool.tile([D_AUG, 4 * P], MDT, tag="lxall", name="lxall")
        DMA_ENG.dma_start(lxall[:], lx_ap[:])
        lhall = const_pool.tile([P, 4 * P], MDT, tag="lhall", name="lhall")
        DMA_ENG.dma_start(lhall[:], lh_ap[:])
        lx_tiles = [lxall[:, g * P:(g + 1) * P] for g in range(4)]
        lh_tiles = [lhall[:, g * P:(g + 1) * P] for g in range(4)]
        wd2 = const_pool.tile([P, 1], MDT, tag="wd2")
        DMA_ENG.dma_start(wd2[:], wd2_ap[:])
        # whole input, loaded once; issue via the Pool sequencer (25ns/issue)
        xall = const_pool.tile([D_AUG, T * B], MDT, tag="xall", name="xall")
        nxc = int(os.environ.get("KERNEL_XCHUNKS", "2"))
        xcw = (T * B) // nxc
        x_engs = [DMA_ENG, nc.scalar] if X_SPLIT_ENG else [DMA_ENG]
        for q in range(nxc):
            x_engs[q % len(x_engs)].dma_start(
                xall[:, q * xcw:(q + 1) * xcw], xaug_ap[:, q * xcw:(q + 1) * xcw]
            )

        # --- per-group fused state (GW = GRP*B cols, chain i at i*B) ---
        z_tiles = []   # [group] PSUM [128, 4*GW]: i|f|o|c gate-major blocks
        hs = []        # [group][phase] fp16 [128, GW]
        cs = []        # [group] f32 [128, GW]
        t1s = []
        t2s = []
        for g in range(NG):
            zt = zst_pool.tile([P, 4 * GW], F32, tag=f"z{g}", name=f"z{g}")
            z_tiles.append(zt)
            hps = []
            for ph in range(2):
                t_ = state_pool.tile([P, GW], MDT, tag=f"h{g}_{ph}",
                                     name=f"h{g}_{ph}")
                nc.vector.memset(t_[:], 0.0)
                hps.append(t_)
            hs.append(hps)
            c2 = state_pool.tile([P, GW], SDT, tag=f"c{g}", name=f"c{g}")
            nc.vector.memset(c2[:], 0.0)
            cs.append(c2)
            t1s.append(state_pool.tile([P, GW], SDT, tag=f"t1_{g}",
                                       name=f"t1_{g}"))
            t2s.append(state_pool.tile([P, GW], SDT, tag=f"t2_{g}",
                                       name=f"t2_{g}"))

        def h_read(g, u):
            return hs[g][(u + 1) % 2]

        def h_write(g, u):
            return hs[g][u % 2]

        # One shared y PSUM tile [B, T] (2 banks); chains write disjoint
        # column ranges (their own segments).
        ypsum = ypsum_pool.tile([B, t_steps], F32, tag="yp", name="ypt")

        def y_mm(j, u):
            start, out0, end = chains[j]
            t = start + u
            if not (out0 <= t < end):
                return
            g, i = divmod(j, GRP)
            nc.tensor.matmul(
                ypsum[:, t:t + 1],
                lhsT=h_write(g, u)[:, i * B:(i + 1) * B], rhs=wd2[:],
                start=True, stop=True,
            )

        sig_of_group = [None] * NG
        for u in range(n_steps):
            g_order = list(range(NG))
            if ROT:
                r = u % NG
                g_order = g_order[r:] + g_order[:r]
            for g in g_order:
                z_cur = z_tiles[g]
                hprev = h_read(g, u)
                # Gate-major emission: the sigmoid only needs the i|f|o
                # blocks, so finishing all chains' i/f/o before the c-gate
                # mms lets sigma start ~5 mm-pairs earlier. Each chain/gate's
                # start=True x-mm stays adjacent to its stop=True rec-mm
                # (PSUM accumulation allows one open group per bank).
                xrhs_i = []
                for i in range(GRP):
                    j = g * GRP + i
                    start, out0, end = chains[j]
                    t = min(start + u, t_steps - 1)
                    xrhs_i.append(xall[:, t * B:(t + 1) * B])
                for gt in range(4):
                    for i in range(GRP):
                        zg = z_cur[:, gt * GW + i * B: gt * GW + (i + 1) * B]
                        nc.tensor.matmul(
                            zg, lhsT=lx_tiles[gt], rhs=xrhs_i[i],
                            start=True, stop=False, skip_group_check=True,
                        )
                        nc.tensor.matmul(
                            zg, lhsT=lh_tiles[gt],
                            rhs=hprev[:, i * B:(i + 1) * B],
                            start=False, stop=True, skip_group_check=True,
                        )
                if u > 0:
                    for i in range(GRP):
                        y_mm(g * GRP + i, u - 1)

                # fused elementwise for the whole group
                sig = sig_pools[g].tile([P, 3 * GW], SDT, tag="sig",
                                        name=f"sig{g}_{u}")
                if SIG_SPLIT:
                    nc.scalar.activation(
                        sig[:, 0:2 * GW], z_cur[:, 0:2 * GW], SIGMOID
                    )
                    nc.scalar.activation(
                        sig[:, 2 * GW:3 * GW], z_cur[:, 2 * GW:3 * GW], SIGMOID
                    )
                else:
                    nc.scalar.activation(sig[:], z_cur[:, 0:3 * GW], SIGMOID)
                # t1 = relu(zc) * sig_i   (the only PSUM-reading DVE op)
                nc.vector.scalar_tensor_tensor(
                    out=t1s[g][:], in0=z_cur[:, 3 * GW:4 * GW], scalar=0.0,
                    in1=sig[:, 0:GW], op0=AluOpType.max, op1=AluOpType.mult,
                )
                # t2 = sig_f * c
                nc.gpsimd.tensor_tensor(
                    out=t2s[g][:], in0=sig[:, GW:2 * GW], in1=cs[g][:],
                    op=AluOpType.mult,
                )
                sig_of_group[g] = sig
                if not TWO_PASS:
                    add_eng = nc.vector if g < ADD_DVE_GROUPS else nc.gpsimd
                    add_eng.tensor_add(cs[g][:], t1s[g][:], t2s[g][:])
                    h_eng = nc.gpsimd if g < H_POOL_GROUPS else nc.vector
                    h_eng.tensor_tensor(
                        out=h_write(g, u)[:], in0=sig[:, 2 * GW:3 * GW],
                        in1=cs[g][:], op=AluOpType.mult,
                    )
            # second pass: adds and h ops, so one group's Pool-side t2
            # can't head-of-line block the next group's ready t1 on the
            # DVE sequencer (stock-op waits block the whole SEQ)
            if TWO_PASS:
                for g in range(NG):
                    sig = sig_of_group[g]
                    add_eng = nc.vector if g < ADD_DVE_GROUPS else nc.gpsimd
                    add_eng.tensor_add(cs[g][:], t1s[g][:], t2s[g][:])
                    nc.vector.tensor_tensor(
                        out=h_write(g, u)[:], in0=sig[:, 2 * GW:3 * GW],
                        in1=cs[g][:], op=AluOpType.mult,
                    )
        for j in range(NCH):
            y_mm(j, n_steps - 1)

        # stage y to SBUF and DMA out
        ysb = out_pool.tile([B, t_steps], F32, tag="ysb", name="ysb")
        nc.scalar.copy(ysb[:], ypsum[:])
        nc.sync.dma_start(y_ap[:], ysb[:])

    nc.compile()
    return nc


def kernel(x, W, U_rec, b, Wd, bd):
    x = np.asarray(x, np.float32)
    W = np.asarray(W, np.float32)
    U_rec = np.asarray(U_rec, np.float32)
    b = np.asarray(b, np.float32)
    Wd = np.asarray(Wd, np.float32)
    bd = np.asarray(bd, np.float32)

    in_maps = _build_core_inputs(x, W, U_rec, b, Wd)
    nc = _build_program(T)
    res = run_bass_kernel_spmd(nc, in_maps, core_ids=list(range(NCORES)))
    ysum = np.zeros((B, T), np.float64)
    for r in res.results:
        ysum += r["y"].astype(np.float64)
    y = (ysum / K + bd[0]).astype(np.float32)
    return y[:, :, None]


if __name__ == "__main__":
    rng = np.random.default_rng(0)
    out = kernel(
        rng.standard_normal((B, T, D), np.float32),
        rng.standard_normal((K, D, 4 * U), np.float32) * 0.05,
        rng.standard_normal((K, U, 4 * U), np.float32) * 0.05,
        np.zeros((K, 4 * U), np.float32),
        rng.standard_normal((U, 1), np.float32) * 0.05,
        np.zeros((1,), np.float32),
    )
    print(out.shape, out.dtype)


# revision 45
# speedup vs baseline: 1.0149x; 1.0149x over previous
"""DeepFactor (K relu-LSTM branches + shared Dense head) on 8 trn2 NeuronCores.

Sharding: the K=10 factor branches are expert-split across cores, 2 slots
per core (16 slots = 10 real + 6 zero-padded; zero weights keep the padded
slot's state identically 0 so padding is exact). Every core runs the same
SPMD program over the full batch B=32.

Time-segment parallelism: the recurrence is latency-bound (the per-step
h -> matmul -> sigmoid -> gates -> h chain is ~1.5-2us while every engine
is far from busy), so each core runs SEGS independent time segments
concurrently. The LSTM is strongly contractive (unit forget bias => forget
gate ~0.7, so state memory decays ~0.7^t): segment j>0 restarts from zero
state WARMUP steps early, and after the warmup its trajectory has converged
to the exact one (L=32 => rel err ~1e-4 measured against the fp32
reference; tol is 2e-2). Serial chain length drops from T to T/SEGS+WARMUP.

Group fusion: chains are fused in groups of GRP for the elementwise ops.
A group shares one PSUM z tile laid out gate-major across chains
([i*GRP | f*GRP | o*GRP | c*GRP] blocks of 32 batch cols each), so the
sigmoid / t1 / t2 / add / h ops each process GRP chains in ONE instruction,
amortizing the per-op fixed costs (DVE access-init 60-125ns, Pool Q7
launch 95ns, ACT init 185ns, SEQ decode) across GRP chains.

Engine split per group-step:
  PE :  z_g = LX_g.T @ [x_t;1] (start) + LH_g.T @ h (stop), per gate/chain
  ACT:  sig = sigmoid(z[:, i|f|o blocks])   exact, one op per group
  DVE:  t1  = max(zc, 0) * sig_i            scalar_tensor_tensor, one op
  Pool: t2  = sig_f * c                     one op
  P/D :  c'  = t1 + t2                      add split across Pool/DVE
  DVE:  h   = sig_o * c'                    relu(c') == c' since c' >= 0
  PE :  y_t = h_j.T @ [Wd;Wd]               per chain, one PSUM column
All chains run exactly T/SEGS + WARMUP steps (chain 0 runs WARMUP extra
steps at its tail instead of a head warmup; y is only emitted for steps
inside the chain's own output segment). Host: y = (sum of cores)/K + bd.
"""

import os
from contextlib import ExitStack

import numpy as np

import concourse.bass as bass
import concourse.tile as tile
from concourse import bacc, mybir
from concourse.alu_op_type import AluOpType
from concourse.bass_utils import run_bass_kernel_spmd

# Problem dims (hardcoded per contract)
B, T, D, U, K = 32, 1024, 32, 64, 10
NCORES = 8

FP16 = os.environ.get("KERNEL_FP16", "1") == "1"
SEGS = int(os.environ.get("KERNEL_SEGS", "20"))      # parallel time segments
GRP = int(os.environ.get("KERNEL_GRP", "4"))         # chains fused per group
WARMUP = int(os.environ.get("KERNEL_WARMUP", "15"))  # zero-state warmup steps
SIG_BUFS = int(os.environ.get("KERNEL_SIG_BUFS", "2"))
# split the sigmoid into i|f (critical: feeds t1/t2) and o (feeds h, late)
SIG_SPLIT = os.environ.get("KERNEL_SIG_SPLIT", "0") == "1"
# fp16 elementwise state (sig/c/t1/t2): halves DVE cycles via 2x mode
FP16_STATE = os.environ.get("KERNEL_FP16_STATE", "1") == "1"
ROT = os.environ.get("KERNEL_ROT", "0") == "1"  # rotate group emission order
TWO_PASS = os.environ.get("KERNEL_TWO_PASS", "0") == "1"
H_POOL_GROUPS = int(os.environ.get("KERNEL_H_POOL_GROUPS", "0"))
Y_DIRECT = os.environ.get("KERNEL_Y_DIRECT", "0") == "1"  # DMA y straight from PSUM
X_SPLIT_ENG = os.environ.get("KERNEL_X_SPLIT_ENG", "1") == "1"  # x chunks on 2 DMA paths  # defer add/h ops
# number of groups whose c'=t1+t2 add runs on DVE instead of Pool (balance)
ADD_DVE_GROUPS = int(os.environ.get("KERNEL_ADD_DVE_GROUPS", "5"))

D_AUG = D + 1  # x rows + bias row

# gate order in the reference weights (Keras): i|f|c|o
_REF_GATE_SLICE = {"i": 0, "f": 1, "c": 2, "o": 3}
# our gate order: i|f|o (sigmoid block) then c (relu'd candidate)
_OUR_GATES = ["i", "f", "o", "c"]


def _np_dt():
    return np.float16 if FP16 else np.float32


def _mm_dt():
    return mybir.dt.float16 if FP16 else mybir.dt.float32


def _segments(t_steps):
    """Per-chain (start, out0, end). All chains run the same number of
    steps n = seg + WARMUP. Chains j>0 warm up from zero state for WARMUP
    steps before their output segment; chain 0 needs no warmup so its
    output segment is the full n steps (y emission is masked outside
    [out0, end))."""
    seg = -(-(t_steps - WARMUP) // SEGS)
    n = seg + WARMUP
    chains = []
    pos = 0
    for j in range(SEGS):
        out0 = pos
        end = min(t_steps, out0 + (n if j == 0 else seg))
        if out0 >= end:
            continue
        start = max(0, out0 - WARMUP)
        chains.append((start, out0, end))
        pos = end
    assert pos >= t_steps
    return chains, n


def _build_core_inputs(x, W, U_rec, b, Wd):
    """Per-core numpy input dicts. Slot assignment: core0:(k0,k1), core1:(k2,k3),
    cores 2-7: (k4+i, pad)."""
    ndt = _np_dt()
    xt = np.ascontiguousarray(np.transpose(x, (2, 1, 0)).reshape(D, T * B))
    xaug = np.concatenate(
        [xt, np.ones((1, T * B), np.float32)], axis=0
    ).astype(ndt)

    slot_ks = [(0, 1), (2, 3)] + [(4 + i, None) for i in range(6)]

    in_maps = []
    for core in range(NCORES):
        ks = slot_ks[core]
        LX = np.zeros((4, D_AUG, 2 * U), np.float32)  # [gate, 33, 128]
        LH = np.zeros((4, 2 * U, 2 * U), np.float32)  # [gate, 128, 128] blockdiag
        WD2 = np.zeros((2 * U, 1), np.float32)
        for s, k in enumerate(ks):
            if k is None:
                continue
            for g, gname in enumerate(_OUR_GATES):
                ref_g = _REF_GATE_SLICE[gname]
                cols = slice(ref_g * U, (ref_g + 1) * U)
                LX[g, :D, s * U:(s + 1) * U] = W[k][:, cols]
                LX[g, D, s * U:(s + 1) * U] = b[k][cols]
                LH[g, s * U:(s + 1) * U, s * U:(s + 1) * U] = U_rec[k][:, cols]
            WD2[s * U:(s + 1) * U, 0] = Wd[:, 0]
        in_maps.append(
            {
                "xaug": xaug,
                "lx": np.ascontiguousarray(
                    np.concatenate(list(LX), axis=1).astype(ndt)),
                "lh": np.ascontiguousarray(
                    np.concatenate(list(LH), axis=1).astype(ndt)),
                "wd2": WD2.astype(ndt),
            }
        )
    return in_maps


def _build_program(t_steps: int) -> bacc.Bacc:
    nc = bacc.Bacc(
        "TRN2",
        target_bir_lowering=False,
        debug=False,
        enable_asserts=False,
        num_devices=NCORES,
    )
    MDT = _mm_dt()
    F32 = mybir.dt.float32
    SDT = mybir.dt.float16 if FP16_STATE else F32
    SIGMOID = mybir.ActivationFunctionType.Sigmoid
    xaug_ap = nc.dram_tensor("xaug", [D_AUG, T * B], MDT, kind="ExternalInput").ap()
    lx_ap = nc.dram_tensor("lx", [D_AUG, 4 * 2 * U], MDT, kind="ExternalInput").ap()
    lh_ap = nc.dram_tensor("lh", [2 * U, 4 * 2 * U], MDT, kind="ExternalInput").ap()
    wd2_ap = nc.dram_tensor("wd2", [2 * U, 1], MDT, kind="ExternalInput").ap()
    y_ap = nc.dram_tensor("y", [B, t_steps], F32, kind="ExternalOutput").ap()

    P = 2 * U  # 128
    chains, n_steps = _segments(t_steps)
    NCH = len(chains)
    assert NCH == SEGS, "partial tail segment not supported by grouping"
    NG = -(-NCH // GRP)
    GW = GRP * B          # fused elementwise width per group

    with tile.TileContext(nc) as tc, ExitStack() as ctx:
        const_pool = ctx.enter_context(tc.tile_pool(name="const", bufs=1))
        state_pool = ctx.enter_context(tc.tile_pool(name="state", bufs=1))
        zst_pool = ctx.enter_context(
            tc.tile_pool(name="zst", bufs=1, space="PSUM")
        )
        sig_pools = [
            ctx.enter_context(tc.tile_pool(name=f"sg{g}", bufs=SIG_BUFS))
            for g in range(NG)
        ]
        ypsum_pool = ctx.enter_context(tc.tile_pool(name="yps", bufs=1, space="PSUM"))
        out_pool = ctx.enter_context(tc.tile_pool(name="out", bufs=1))

        # --- static weights + full x into SBUF ---
        # Weights first: every matmul needs them, and the serialized DMA
        # channel delivers in issue order (x-first measurably regresses).
        DMA_ENG = {"gpsimd": nc.gpsimd, "sync": nc.sync, "scalar": nc.scalar}[
            os.environ.get("KERNEL_DMA_ENG", "gpsimd")
        ]
        # gates packed side by side: one DMA for all x-weights, one for all
        # rec-weights. The two DMA initiation paths (gpsimd SWDGE / ACT
        # HWDGE) run in parallel, so balance bytes across them: big lh +
        # one x half on path A, small lx + wd2 + other x half on path B.
        lxall = const_pool.tile([D_AUG, 4 * P], MDT, tag="lxall", name="lxall")
        DMA_ENG.dma_start(lxall[:], lx_ap[:])
        lhall = const_pool.tile([P, 4 * P], MDT, tag="lhall", name="lhall")
        DMA_ENG.dma_start(lhall[:], lh_ap[:])
        lx_tiles = [lxall[:, g * P:(g + 1) * P] for g in range(4)]
        lh_tiles = [lhall[:, g * P:(g + 1) * P] for g in range(4)]
        wd2 = const_pool.tile([P, 1], MDT, tag="wd2")
        DMA_ENG.dma_start(wd2[:], wd2_ap[:])
        # whole input, loaded once; issue via the Pool sequencer (25ns/issue)
        xall = const_pool.tile([D_AUG, T * B], MDT, tag="xall", name="xall")
        nxc = int(os.environ.get("KERNEL_XCHUNKS", "2"))
        xcw = (T * B) // nxc
        x_engs = [nc.gpsimd, nc.scalar] if X_SPLIT_ENG else [DMA_ENG]
        for q in range(nxc):
            x_engs[q % len(x_engs)].dma_start(
                xall[:, q * xcw:(q + 1) * xcw], xaug_ap[:, q * xcw:(q + 1) * xcw]
            )

        # --- per-group fused state (GW = GRP*B cols, chain i at i*B) ---
        z_tiles = []   # [group] PSUM [128, 4*GW]: i|f|o|c gate-major blocks
        hs = []        # [group][phase] fp16 [128, GW]
        cs = []        # [group] f32 [128, GW]
        t1s = []
        t2s = []
        for g in range(NG):
            zt = zst_pool.tile([P, 4 * GW], F32, tag=f"z{g}", name=f"z{g}")
            z_tiles.append(zt)
            hps = []
            for ph in range(2):
                t_ = state_pool.tile([P, GW], MDT, tag=f"h{g}_{ph}",
                                     name=f"h{g}_{ph}")
                nc.vector.memset(t_[:], 0.0)
                hps.append(t_)
            hs.append(hps)
            c2 = state_pool.tile([P, GW], SDT, tag=f"c{g}", name=f"c{g}")
            nc.vector.memset(c2[:], 0.0)
            cs.append(c2)
            t1s.append(state_pool.tile([P, GW], SDT, tag=f"t1_{g}",
                                       name=f"t1_{g}"))
            t2s.append(state_pool.tile([P, GW], SDT, tag=f"t2_{g}",
                                       name=f"t2_{g}"))

        def h_read(g, u):
            return hs[g][(u + 1) % 2]

        def h_write(g, u):
            return hs[g][u % 2]

        # One shared y PSUM tile [B, T] (2 banks); chains write disjoint
        # column ranges (their own segments).
        ypsum = ypsum_pool.tile([B, t_steps], F32, tag="yp", name="ypt")

        def y_mm(j, u):
            start, out0, end = chains[j]
            t = start + u
            if not (out0 <= t < end):
                return
            g, i = divmod(j, GRP)
            nc.tensor.matmul(
                ypsum[:, t:t + 1],
                lhsT=h_write(g, u)[:, i * B:(i + 1) * B], rhs=wd2[:],
                start=True, stop=True,
            )

        sig_of_group = [None] * NG
        for u in range(n_steps):
            g_order = list(range(NG))
            if ROT:
                r = u % NG
                g_order = g_order[r:] + g_order[:r]
            for g in g_order:
                z_cur = z_tiles[g]
                hprev = h_read(g, u)
                # Gate-major emission: the sigmoid only needs the i|f|o
                # blocks, so finishing all chains' i/f/o before the c-gate
                # mms lets sigma start ~5 mm-pairs earlier. Each chain/gate's
                # start=True x-mm stays adjacent to its stop=True rec-mm
                # (PSUM accumulation allows one open group per bank).
                xrhs_i = []
                for i in range(GRP):
                    j = g * GRP + i
                    start, out0, end = chains[j]
                    t = min(start + u, t_steps - 1)
                    xrhs_i.append(xall[:, t * B:(t + 1) * B])
                for gt in range(4):
                    for i in range(GRP):
                        zg = z_cur[:, gt * GW + i * B: gt * GW + (i + 1) * B]
                        nc.tensor.matmul(
                            zg, lhsT=lx_tiles[gt], rhs=xrhs_i[i],
                            start=True, stop=False, skip_group_check=True,
                        )
                        nc.tensor.matmul(
                            zg, lhsT=lh_tiles[gt],
                            rhs=hprev[:, i * B:(i + 1) * B],
                            start=False, stop=True, skip_group_check=True,
                        )
                if u > 0:
                    for i in range(GRP):
                        y_mm(g * GRP + i, u - 1)

                # fused elementwise for the whole group
                sig = sig_pools[g].tile([P, 3 * GW], SDT, tag="sig",
                                        name=f"sig{g}_{u}")
                if SIG_SPLIT:
                    nc.scalar.activation(
                        sig[:, 0:2 * GW], z_cur[:, 0:2 * GW], SIGMOID
                    )
                    nc.scalar.activation(
                        sig[:, 2 * GW:3 * GW], z_cur[:, 2 * GW:3 * GW], SIGMOID
                    )
                else:
                    nc.scalar.activation(sig[:], z_cur[:, 0:3 * GW], SIGMOID)
                # t1 = relu(zc) * sig_i   (the only PSUM-reading DVE op)
                nc.vector.scalar_tensor_tensor(
                    out=t1s[g][:], in0=z_cur[:, 3 * GW:4 * GW], scalar=0.0,
                    in1=sig[:, 0:GW], op0=AluOpType.max, op1=AluOpType.mult,
                )
                # t2 = sig_f * c
                nc.gpsimd.tensor_tensor(
                    out=t2s[g][:], in0=sig[:, GW:2 * GW], in1=cs[g][:],
                    op=AluOpType.mult,
                )
                sig_of_group[g] = sig
                if not TWO_PASS:
                    add_eng = nc.vector if g < ADD_DVE_GROUPS else nc.gpsimd
                    add_eng.tensor_add(cs[g][:], t1s[g][:], t2s[g][:])
                    h_eng = nc.gpsimd if g < H_POOL_GROUPS else nc.vector
                    h_eng.tensor_tensor(
                        out=h_write(g, u)[:], in0=sig[:, 2 * GW:3 * GW],
                        in1=cs[g][:], op=AluOpType.mult,
                    )
            # second pass: adds and h ops, so one group's Pool-side t2
            # can't head-of-line block the next group's ready t1 on the
            # DVE sequencer (stock-op waits block the whole SEQ)
            if TWO_PASS:
                for g in range(NG):
                    sig = sig_of_group[g]
                    add_eng = nc.vector if g < ADD_DVE_GROUPS else nc.gpsimd
                    add_eng.tensor_add(cs[g][:], t1s[g][:], t2s[g][:])
                    nc.vector.tensor_tensor(
                        out=h_write(g, u)[:], in0=sig[:, 2 * GW:3 * GW],
                        in1=cs[g][:], op=AluOpType.mult,
                    )
        for j in range(NCH):
            y_mm(j, n_steps - 1)

        # stage y to SBUF and DMA out
        ysb = out_pool.tile([B, t_steps], F32, tag="ysb", name="ysb")
        nc.scalar.copy(ysb[:], ypsum[:])
        nc.sync.dma_start(y_ap[:], ysb[:])

    nc.compile()
    return nc


def kernel(x, W, U_rec, b, Wd, bd):
    x = np.asarray(x, np.float32)
    W = np.asarray(W, np.float32)
    U_rec = np.asarray(U_rec, np.float32)
    b = np.asarray(b, np.float32)
    Wd = np.asarray(Wd, np.float32)
    bd = np.asarray(bd, np.float32)

    in_maps = _build_core_inputs(x, W, U_rec, b, Wd)
    nc = _build_program(T)
    res = run_bass_kernel_spmd(nc, in_maps, core_ids=list(range(NCORES)))
    ysum = np.zeros((B, T), np.float64)
    for r in res.results:
        ysum += r["y"].astype(np.float64)
    y = (ysum / K + bd[0]).astype(np.float32)
    return y[:, :, None]


if __name__ == "__main__":
    rng = np.random.default_rng(0)
    out = kernel(
        rng.standard_normal((B, T, D), np.float32),
        rng.standard_normal((K, D, 4 * U), np.float32) * 0.05,
        rng.standard_normal((K, U, 4 * U), np.float32) * 0.05,
        np.zeros((K, 4 * U), np.float32),
        rng.standard_normal((U, 1), np.float32) * 0.05,
        np.zeros((1,), np.float32),
    )
    print(out.shape, out.dtype)
